# revision 2
# baseline (speedup 1.0000x reference)
"""Distributed multi-head causal attention for 8 TRN2 NeuronCores.

Problem: B=4, T=2048, D=2048, H=16 heads of dk=dv=128.
  out = softmax(mask((q@Wq)(k@Wk)^T / sqrt(dk))) @ (v@Wv) @ Wo

Sharding (2D, all per-core asymmetry lives in host-supplied data so the
SPMD graph is identical on all cores):
  core c -> batch b = c//2, head-group g = c%2 (heads 8g..8g+7).
  - QKV projections + attention for (batch b, its 8 heads): fully local.
  - Pair AllGather (replica groups [2b, 2b+1]) exchanges the per-head
    attention outputs (merged^T, bf16), chunked by q-512 for overlap.
  - Output projection: each core computes out^T for its batch for HALF
    the output columns (even core: cols 0..1023, odd: 1024..2047).
  Host reassembles: out[b] = concat(outT_2b, outT_2b+1, axis=0).T

Compute is bf16 on TensorE with f32 PSUM accumulation. Softmax skips the
max-subtraction (scores are ~N(0,1); exp is safe in f32) and obtains the
denominators with an extra ones-matmul so everything stays on TensorE;
masking multiplies exp(scores) by 0/1 tiles after the fact.

Layouts per core (all bf16 unless noted):
  qT/kT/vT [D=2048, T=2048]   x[b].T            (contraction on partitions)
  wq/wk/wv [D=2048, 1024]     W[:, 1024g:1024(g+1)]
  wo       [2048, 1024]       Wo[:, 1024g:1024(g+1)]
  masks    [4, 128, 512]      causal shift masks, masks[i, k, j] = j >= k + 128*i
  maskT    [2048, 2048]       general mode: mask.T (0/1)
  outT     [1024, 2048] f32   out[b][:, cols].T
Internal: q_s/k_s [8, 128, 2048] (Q^T/K^T per head), v_s [8, 16, 128, 128]
  (V natural, per head per k-block), cc_in/cc_out per q-chunk.
"""
import os
import sys

import numpy as np
import ml_dtypes

import concourse.bass as bass
import concourse.mybir as mybir
import concourse.tile as tile
from concourse import bacc
from concourse.bass_utils import run_bass_kernel_spmd

BF16 = mybir.dt.bfloat16
F32 = mybir.dt.float32

B, T, D = 4, 2048, 2048
H, DK, DV = 16, 128, 128
HG = 8                      # heads per core
N_CORES = 8
QC = 512                    # q-chunk (matmul moving free dim)
NQC = T // QC               # 4
NKB = T // 128              # 16 k-blocks
NDC = D // 128              # 16 contraction chunks
SCALE = 1.0 / np.sqrt(DK)

_KERNEL_CACHE = {}


def build_kernel(causal: bool):
    nc = bacc.Bacc("TRN2", num_devices=N_CORES)

    qT = nc.declare_dram_parameter("qT", [D, T], BF16, isOutput=False)
    kT = nc.declare_dram_parameter("kT", [D, T], BF16, isOutput=False)
    vT = nc.declare_dram_parameter("vT", [D, T], BF16, isOutput=False)
    wq = nc.declare_dram_parameter("wq", [D, HG * DK], BF16, isOutput=False)
    wk = nc.declare_dram_parameter("wk", [D, HG * DK], BF16, isOutput=False)
    wv = nc.declare_dram_parameter("wv", [D, HG * DV], BF16, isOutput=False)
    wo = nc.declare_dram_parameter("wo", [H * DV, D // 2], BF16, isOutput=False)
    if causal:
        masks = nc.declare_dram_parameter("masks", [4, 128, QC], BF16, isOutput=False)
    else:
        maskT = nc.declare_dram_parameter("maskT", [T, T], BF16, isOutput=False)
    outT = nc.declare_dram_parameter("outT", [D // 2, T], F32, isOutput=True)

    q_s = nc.dram_tensor("q_s", [HG, 128, T], BF16)
    k_s = nc.dram_tensor("k_s", [HG, 128, T], BF16)
    v_s = nc.dram_tensor("v_s", [HG, NKB, 128, DV], BF16)
    cc_in = [nc.dram_tensor(f"cc_in_{j}", [128, HG, QC], BF16) for j in range(NQC)]
    # NB: 2-core replica groups don't support Shared-output collectives;
    # Local output is the supported path there (see replica_groups.py).
    cc_out = [
        nc.dram_tensor(f"cc_out_{j}", [2, 128, HG, QC], BF16)
        for j in range(NQC)
    ]
    pair_groups = [[0, 1], [2, 3], [4, 5], [6, 7]]

    with tile.TileContext(nc) as tc:
        with tc.tile_pool(name="consts", bufs=1) as consts:
            # Constants up front: Wo (used at the end; DMA hides under compute),
            # the causal shift masks, and the all-ones stationary operand used
            # to compute softmax denominators on TensorE.
            wo_sb = consts.tile([128, NDC, D // 2], BF16)
            nc.sync.dma_start(out=wo_sb, in_=wo.rearrange("(o p) f -> p o f", p=128))
            ones_sb = consts.tile([128, 128], BF16)
            nc.vector.memset(ones_sb, 1.0)
            if causal:
                mask_sb = consts.tile([128, 4, QC], BF16)
                nc.sync.dma_start(out=mask_sb, in_=masks.rearrange("o p f -> p o f"))

            # ---------------- Phase 1: QKV projections ----------------
            # Q^T/K^T per head ([dk, q] with dk on partitions) and V natural
            # ([krows, dv] with krows on partitions), staged through DRAM.
            with (
                tc.tile_pool(name="xt", bufs=3) as xt_pool,
                tc.tile_pool(name="w", bufs=2) as w_pool,
                tc.tile_pool(name="pstage", bufs=3) as pstage,
                tc.tile_pool(name="vstage", bufs=2) as vstage,
                tc.tile_pool(name="ppsum", bufs=2, space="PSUM") as ppsum,
                tc.tile_pool(name="vpsum", bufs=2, space="PSUM") as vpsum,
            ):
                for x_ext, w_ext, kind in ((qT, wq, "q"), (kT, wk, "k"), (vT, wv, "v")):
                    xh = []
                    wh = []
                    for half in range(2):
                        xtile = xt_pool.tile([128, NDC // 2, T], BF16, tag="xt")
                        nc.sync.dma_start(
                            out=xtile,
                            in_=x_ext[half * 1024 : (half + 1) * 1024].rearrange(
                                "(o p) f -> p o f", p=128
                            ),
                        )
                        xh.append(xtile)
                        wtile = w_pool.tile([128, NDC // 2, HG * 128], BF16, tag="w")
                        nc.sync.dma_start(
                            out=wtile,
                            in_=w_ext[half * 1024 : (half + 1) * 1024].rearrange(
                                "(o p) f -> p o f", p=128
                            ),
                        )
                        wh.append(wtile)

                    if kind in ("q", "k"):
                        dst = q_s if kind == "q" else k_s
                        for h in range(HG):
                            for qc in range(NQC):
                                ps = ppsum.tile([128, QC], F32, tag="ppsum")
                                for half in range(2):
                                    for dci in range(NDC // 2):
                                        nc.tensor.matmul(
                                            ps,
                                            lhsT=wh[half][:, dci, h * 128 : (h + 1) * 128],
                                            rhs=xh[half][:, dci, qc * QC : (qc + 1) * QC],
                                            start=(half == 0 and dci == 0),
                                            stop=(half == 1 and dci == NDC // 2 - 1),
                                        )
                                sb = pstage.tile([128, QC], BF16, tag="pstage")
                                nc.vector.tensor_copy(out=sb, in_=ps)
                                nc.sync.dma_start(
                                    out=dst[h, :, qc * QC : (qc + 1) * QC], in_=sb
                                )
                    else:
                        for kb in range(NKB):
                            ps = vpsum.tile([128, HG * DV], F32, tag="vpsum")
                            for half in range(2):
                                for dci in range(NDC // 2):
                                    lhsT = xh[half][:, dci, kb * 128 : (kb + 1) * 128]
                                    for nn in range(2):
                                        nc.tensor.matmul(
                                            ps[:, nn * 512 : (nn + 1) * 512],
                                            lhsT=lhsT,
                                            rhs=wh[half][:, dci, nn * 512 : (nn + 1) * 512],
                                            start=(half == 0 and dci == 0),
                                            stop=(half == 1 and dci == NDC // 2 - 1),
                                        )
                            sb = vstage.tile([128, HG * DV], BF16, tag="vstage")
                            nc.vector.tensor_copy(out=sb, in_=ps)
                            for h in range(HG):
                                nc.sync.dma_start(
                                    out=v_s[h, kb],
                                    in_=sb[:, h * DV : (h + 1) * DV],
                                )

            # ---------- Phase 2+3: attention, pair-AG, output proj ----------
            with (
                tc.tile_pool(name="qh", bufs=2) as qh_pool,
                tc.tile_pool(name="kh", bufs=2) as kh_pool,
                tc.tile_pool(name="vh", bufs=2) as vh_pool,
                tc.tile_pool(name="pt", bufs=4) as pt_pool,
                tc.tile_pool(name="mstage", bufs=3) as mstage,
                tc.tile_pool(name="rinv", bufs=2) as rinv_pool,
                tc.tile_pool(name="mf", bufs=2) as mf_pool,
                tc.tile_pool(name="ob", bufs=3) as ob_pool,
                tc.tile_pool(name="spsum", bufs=2, space="PSUM") as spsum,
                tc.tile_pool(name="opsum", bufs=2, space="PSUM") as opsum,
                tc.tile_pool(name="rpsum", bufs=2, space="PSUM") as rpsum,
                tc.tile_pool(name="wpsum", bufs=2, space="PSUM") as wpsum,
                (
                    tc.tile_pool(name="gm", bufs=2)
                    if not causal
                    else _null_ctx()
                ) as gm_pool,
            ):
                # Big q-chunks first so the last AllGather (smallest chunk)
                # has the least compute left to hide behind.
                for qc in (3, 2, 1, 0):
                    nkb = 4 * (qc + 1) if causal else NKB
                    if not causal:
                        gm = gm_pool.tile([128, NKB, QC], BF16, tag="gm")
                        nc.sync.dma_start(
                            out=gm,
                            in_=maskT[:, qc * QC : (qc + 1) * QC].rearrange(
                                "(o p) f -> p o f", p=128
                            ),
                        )
                    for h in range(HG):
                        qh = qh_pool.tile([128, QC], BF16, tag="qh")
                        nc.sync.dma_start(out=qh, in_=q_s[h, :, qc * QC : (qc + 1) * QC])
                        kh = kh_pool.tile([128, T], BF16, tag="kh")
                        nc.sync.dma_start(
                            out=kh[:, : nkb * 128], in_=k_s[h, :, : nkb * 128]
                        )
                        vh = vh_pool.tile([128, NKB, DV], BF16, tag="vh")
                        nc.sync.dma_start(
                            out=vh[:, :nkb, :],
                            in_=v_s[h, :nkb].rearrange("o p f -> p o f"),
                        )
                        o_ps = opsum.tile([128, QC], F32, tag="opsum")
                        r_ps = rpsum.tile([128, QC], F32, tag="rpsum")
                        for kb in range(nkb):
                            s_ps = spsum.tile([128, QC], F32, tag="spsum")
                            nc.tensor.matmul(
                                s_ps,
                                lhsT=kh[:, kb * 128 : (kb + 1) * 128],
                                rhs=qh,
                                start=True,
                                stop=True,
                            )
                            pt = pt_pool.tile([128, QC], BF16, tag="pt")
                            nc.scalar.activation(
                                out=pt,
                                in_=s_ps,
                                func=mybir.ActivationFunctionType.Exp,
                                scale=float(SCALE),
                            )
                            if causal:
                                shift = kb - 4 * qc
                                if shift >= 0:
                                    nc.vector.tensor_mul(
                                        out=pt, in0=pt, in1=mask_sb[:, shift, :]
                                    )
                            else:
                                nc.vector.tensor_mul(out=pt, in0=pt, in1=gm[:, kb, :])
                            nc.tensor.matmul(
                                o_ps,
                                lhsT=vh[:, kb, :],
                                rhs=pt,
                                start=(kb == 0),
                                stop=(kb == nkb - 1),
                            )
                            nc.tensor.matmul(
                                r_ps,
                                lhsT=ones_sb,
                                rhs=pt,
                                start=(kb == 0),
                                stop=(kb == nkb - 1),
                            )
                        rinv = rinv_pool.tile([128, QC], F32, tag="rinv")
                        nc.vector.reciprocal(out=rinv, in_=r_ps)
                        msb = mstage.tile([128, QC], BF16, tag="mstage")
                        nc.vector.tensor_mul(out=msb, in0=o_ps, in1=rinv)
                        nc.sync.dma_start(out=cc_in[qc][:, h, :], in_=msb)

                    nc.gpsimd.collective_compute(
                        "AllGather",
                        mybir.AluOpType.bypass,
                        ins=[cc_in[qc][:]],
                        outs=[cc_out[qc][:]],
                        replica_groups=pair_groups,
                    )

                    # Output projection for this q-chunk (runs once the pair
                    # exchange lands; overlaps the next q-chunk's attention).
                    mf = mf_pool.tile([128, H, QC], BF16, tag="mf")
                    nc.sync.dma_start(out=mf[:, :HG, :], in_=cc_out[qc][0])
                    nc.sync.dma_start(out=mf[:, HG:, :], in_=cc_out[qc][1])
                    for col in range(D // 2 // 128):
                        w_ps = wpsum.tile([128, QC], F32, tag="wpsum")
                        for hv in range(H):
                            nc.tensor.matmul(
                                w_ps,
                                lhsT=wo_sb[:, hv, col * 128 : (col + 1) * 128],
                                rhs=mf[:, hv, :],
                                start=(hv == 0),
                                stop=(hv == H - 1),
                            )
                        ob = ob_pool.tile([128, QC], F32, tag="ob")
                        nc.vector.tensor_copy(out=ob, in_=w_ps)
                        nc.sync.dma_start(
                            out=outT[col * 128 : (col + 1) * 128, qc * QC : (qc + 1) * QC],
                            in_=ob,
                        )

    nc.compile()
    return nc


class _null_ctx:
    def __enter__(self):
        return None

    def __exit__(self, *a):
        return False


def _causal_masks() -> np.ndarray:
    i = np.arange(128)[:, None]
    j = np.arange(QC)[None, :]
    m = np.stack([(j >= i + 128 * s) for s in range(4)]).astype(ml_dtypes.bfloat16)
    return m


def kernel(q, k, v, mask, Wq, Wk, Wv, Wo):
    q = np.asarray(q)
    k = np.asarray(k)
    v = np.asarray(v)
    mask = np.asarray(mask)
    causal = bool(np.array_equal(mask, np.tril(np.ones((T, T), dtype=bool))))

    if causal not in _KERNEL_CACHE:
        _KERNEL_CACHE[causal] = build_kernel(causal)
    nc = _KERNEL_CACHE[causal]

    bf = ml_dtypes.bfloat16
    Wq_b = np.asarray(Wq).astype(bf)
    Wk_b = np.asarray(Wk).astype(bf)
    Wv_b = np.asarray(Wv).astype(bf)
    Wo_b = np.asarray(Wo).astype(bf)
    masks_np = _causal_masks()
    maskT_np = None if causal else np.ascontiguousarray(mask.T).astype(bf)

    in_maps = []
    for c in range(N_CORES):
        b, g = c // 2, c % 2
        m = {
            "qT": np.ascontiguousarray(q[b].T).astype(bf),
            "kT": np.ascontiguousarray(k[b].T).astype(bf),
            "vT": np.ascontiguousarray(v[b].T).astype(bf),
            "wq": np.ascontiguousarray(Wq_b[:, g * 1024 : (g + 1) * 1024]),
            "wk": np.ascontiguousarray(Wk_b[:, g * 1024 : (g + 1) * 1024]),
            "wv": np.ascontiguousarray(Wv_b[:, g * 1024 : (g + 1) * 1024]),
            "wo": np.ascontiguousarray(Wo_b[:, g * 1024 : (g + 1) * 1024]),
        }
        if causal:
            m["masks"] = masks_np
        else:
            m["maskT"] = maskT_np
        in_maps.append(m)

    trace = bool(os.environ.get("BASS_KERNEL_TRACE")) and (
        "antenv.axon_hooks" in sys.modules
    )
    res = run_bass_kernel_spmd(nc, in_maps, list(range(N_CORES)), trace=trace)
    if trace and res.exec_time_ns is not None:
        print(f"HW exec time: {res.exec_time_ns} ns")
        kernel.last_exec_time_ns = res.exec_time_ns
        kernel.last_results = res

    out = np.empty((B, T, D), dtype=np.float32)
    for b in range(B):
        top = res.results[2 * b]["outT"]        # cols 0..1023, [1024, 2048]
        bot = res.results[2 * b + 1]["outT"]    # cols 1024..2047
        out[b] = np.concatenate([top, bot], axis=0).T
    return out


# revision 5
# speedup vs baseline: 1.0897x; 1.0897x over previous
"""Distributed multi-head causal attention for 8 TRN2 NeuronCores.

Problem: B=4, T=2048, D=2048, H=16 heads of dk=dv=128.
  out = softmax(mask((q@Wq)(k@Wk)^T / sqrt(dk))) @ (v@Wv) @ Wo

Sharding (2D; all per-core asymmetry lives in host-supplied data so the
SPMD graph is identical on every core):
  core c -> batch b = c//2, head-group g = c%2 (heads 8g..8g+7).
  - QKV projections + attention for (batch b, its 8 heads): fully local.
  - Pair AllGather (replica groups [2b, 2b+1]) exchanges the per-head
    attention outputs (merged^T, bf16), chunked by q-512 AND head-pair
    (16 small collectives) so the exchange pipelines into the output
    projection.
  - Output projection: each core computes out^T for its batch for HALF
    the output columns (even core: cols 0..1023, odd: 1024..2047); the
    hv-accumulation is split in two halves so the first half starts
    before the last sub-AllGather lands.
  Host reassembles: out[b] = concat(outT_2b, outT_2b+1, axis=0).T

Compute is bf16 on TensorE with f32 PSUM accumulation. Softmax skips the
max-subtraction (scores are ~N(0,1); exp is safe in f32) and obtains the
denominators with an extra ones-matmul so everything stays on TensorE;
causal masking multiplies exp(scores) by a 0/1 triangular tile on the
single diagonal-crossing 128x128 sub-block, and the moving free dim of
diagonal-region matmuls is trimmed to the unmasked columns.

Layouts per core (all bf16 unless noted):
  qT/kT/vT [D=2048, T=2048]   x[b].T            (contraction on partitions)
  wq/wk/wv [D=2048, 1024]     W[:, 1024g:1024(g+1)]
  wo       [2048, 1024]       Wo[:, 1024g:1024(g+1)]
  tri      [128, 128]         causal tile, tri[k, j] = j >= k
  maskT    [2048, 2048]       general mode: mask.T (0/1)
  outT     [1024, 2048] f32   out[b][:, cols].T
Internal: q_s/k_s [8, 128, 2048] (Q^T/K^T per head), v_s [8, 16, 128, 128]
  (V natural, per head per k-block), cc_in/cc_out per (q-chunk, head-pair).
"""
import os
import sys
from contextlib import ExitStack

import numpy as np
import ml_dtypes

import concourse.bass as bass
import concourse.mybir as mybir
import concourse.tile as tile
from concourse import bacc
from concourse.bass_utils import run_bass_kernel_spmd

BF16 = mybir.dt.bfloat16
F32 = mybir.dt.float32

B, T, D = 4, 2048, 2048
H, DK, DV = 16, 128, 128
HG = 8                      # heads per core
N_CORES = 8
QC = 512                    # q-chunk (matmul moving free dim)
NQC = T // QC               # 4
NKB = T // 128              # 16 k-blocks
NDC = D // 128              # 16 contraction chunks
SCALE = 1.0 / np.sqrt(DK)
N_WARM = 150                # dummy matmuls to warm the PE clock gate

_KERNEL_CACHE = {}


class _null_ctx:
    def __enter__(self):
        return None

    def __exit__(self, *a):
        return False


def build_kernel(causal: bool):
    nc = bacc.Bacc("TRN2", num_devices=N_CORES)

    qT = nc.declare_dram_parameter("qT", [D, T], BF16, isOutput=False)
    kT = nc.declare_dram_parameter("kT", [D, T], BF16, isOutput=False)
    vT = nc.declare_dram_parameter("vT", [D, T], BF16, isOutput=False)
    wq = nc.declare_dram_parameter("wq", [D, HG * DK], BF16, isOutput=False)
    wk = nc.declare_dram_parameter("wk", [D, HG * DK], BF16, isOutput=False)
    wv = nc.declare_dram_parameter("wv", [D, HG * DV], BF16, isOutput=False)
    wo = nc.declare_dram_parameter("wo", [H * DV, D // 2], BF16, isOutput=False)
    tri = nc.declare_dram_parameter("tri", [128, 128], BF16, isOutput=False)
    if not causal:
        maskT = nc.declare_dram_parameter("maskT", [T, T], BF16, isOutput=False)
    outT = nc.declare_dram_parameter("outT", [D // 2, T], F32, isOutput=True)

    q_s = nc.dram_tensor("q_s", [HG, 128, T], BF16)
    k_s = nc.dram_tensor("k_s", [HG, 128, T], BF16)
    v_s = nc.dram_tensor("v_s", [HG, NKB, 128, DV], BF16)
    # One small collective per (q-chunk, head-pair): cc_in holds the two
    # local heads 2j, 2j+1; the pair gather adds global heads 8+2j, 9+2j.
    cc_in = [
        [nc.dram_tensor(f"cc_in_{qc}_{j}", [128, 2, QC], BF16) for j in range(4)]
        for qc in range(NQC)
    ]
    # 2-core replica groups don't support Shared-output collectives; Local
    # output is the supported path there (see replica_groups.py).
    cc_out = [
        [nc.dram_tensor(f"cc_out_{qc}_{j}", [2, 128, 2, QC], BF16) for j in range(4)]
        for qc in range(NQC)
    ]
    pair_groups = [[0, 1], [2, 3], [4, 5], [6, 7]]

    def kb_start(qc, kb):
        """First unmasked q column (within the chunk) for this k-block."""
        if not causal:
            return 0
        return min(max((kb - 4 * qc) * 128, 0), QC)

    with tile.TileContext(nc) as tc:
        with tc.tile_pool(name="consts", bufs=1) as consts:
            ones_sb = consts.tile([128, 128], BF16)
            nc.vector.memset(ones_sb, 1.0)
            tri_sb = consts.tile([128, 128], BF16)
            nc.sync.dma_start(out=tri_sb, in_=tri[:])

            # Warm the PE HAM clock gate while the first input DMAs land:
            # dependency-free matmuls on the ones tile into a scratch bank.
            with tc.tile_pool(name="warmps", bufs=1, space="PSUM") as warmps:
                wps = warmps.tile([128, 128], F32)
                for i in range(N_WARM):
                    nc.tensor.matmul(
                        wps, lhsT=ones_sb, rhs=ones_sb,
                        start=(i == 0), stop=(i == N_WARM - 1),
                    )

                # ---------------- Phase 1: QKV projections ----------------
                # Q^T/K^T per head ([dk, q], dk on partitions) and V natural
                # ([krows, dv], krows on partitions), staged through DRAM.
                with (
                    tc.tile_pool(name="xt", bufs=4) as xt_pool,
                    tc.tile_pool(name="w", bufs=2) as w_pool,
                    tc.tile_pool(name="pstage", bufs=3) as pstage,
                    tc.tile_pool(name="vstage", bufs=2) as vstage,
                    tc.tile_pool(name="ppsum", bufs=2, space="PSUM") as ppsum,
                    tc.tile_pool(name="vpsum", bufs=2, space="PSUM") as vpsum,
                ):
                    for x_ext, w_ext, kind in (
                        (qT, wq, "q"), (kT, wk, "k"), (vT, wv, "v")
                    ):
                        xh, wh = [], []
                        for half in range(2):
                            xtile = xt_pool.tile([128, NDC // 2, T], BF16, tag="xt")
                            nc.sync.dma_start(
                                out=xtile,
                                in_=x_ext[half * 1024 : (half + 1) * 1024].rearrange(
                                    "(o p) f -> p o f", p=128
                                ),
                            )
                            xh.append(xtile)
                            wtile = w_pool.tile([128, NDC // 2, HG * 128], BF16, tag="w")
                            nc.sync.dma_start(
                                out=wtile,
                                in_=w_ext[half * 1024 : (half + 1) * 1024].rearrange(
                                    "(o p) f -> p o f", p=128
                                ),
                            )
                            wh.append(wtile)

                        if kind in ("q", "k"):
                            dst = q_s if kind == "q" else k_s
                            for h in range(HG):
                                for qc in range(NQC):
                                    ps = ppsum.tile([128, QC], F32, tag="ppsum")
                                    for half in range(2):
                                        for dci in range(NDC // 2):
                                            nc.tensor.matmul(
                                                ps,
                                                lhsT=wh[half][
                                                    :, dci, h * 128 : (h + 1) * 128
                                                ],
                                                rhs=xh[half][
                                                    :, dci, qc * QC : (qc + 1) * QC
                                                ],
                                                start=(half == 0 and dci == 0),
                                                stop=(half == 1 and dci == NDC // 2 - 1),
                                            )
                                    sb = pstage.tile([128, QC], BF16, tag="pstage")
                                    nc.vector.tensor_copy(out=sb, in_=ps)
                                    nc.sync.dma_start(
                                        out=dst[h, :, qc * QC : (qc + 1) * QC], in_=sb
                                    )
                        else:
                            for kb in range(NKB):
                                ps = vpsum.tile([128, HG * DV], F32, tag="vpsum")
                                for half in range(2):
                                    for dci in range(NDC // 2):
                                        lhsT = xh[half][:, dci, kb * 128 : (kb + 1) * 128]
                                        for nn in range(2):
                                            nc.tensor.matmul(
                                                ps[:, nn * 512 : (nn + 1) * 512],
                                                lhsT=lhsT,
                                                rhs=wh[half][
                                                    :, dci, nn * 512 : (nn + 1) * 512
                                                ],
                                                start=(half == 0 and dci == 0),
                                                stop=(half == 1 and dci == NDC // 2 - 1),
                                            )
                                sb = vstage.tile([128, HG * DV], BF16, tag="vstage")
                                nc.vector.tensor_copy(out=sb, in_=ps)
                                for h in range(HG):
                                    nc.sync.dma_start(
                                        out=v_s[h, kb], in_=sb[:, h * DV : (h + 1) * DV]
                                    )

            # ---------- Phase 2+3: attention, pair-AG, output proj ----------
            with ExitStack() as phase2:
                ent = phase2.enter_context
                wos_pool = ent(tc.tile_pool(name="wos", bufs=1))
                qh_pool = ent(tc.tile_pool(name="qh", bufs=3))
                kh_pool = ent(tc.tile_pool(name="kh", bufs=3))
                vh_pool = ent(tc.tile_pool(name="vh", bufs=3))
                pt_pool = ent(tc.tile_pool(name="pt", bufs=6))
                mstage = ent(tc.tile_pool(name="mstage", bufs=3))
                rinv_pool = ent(tc.tile_pool(name="rinv", bufs=2))
                mf_pool = ent(tc.tile_pool(name="mf", bufs=8))
                wpart_pool = ent(tc.tile_pool(name="wpart", bufs=8))
                ob_pool = ent(tc.tile_pool(name="ob", bufs=3))
                spsum = ent(tc.tile_pool(name="spsum", bufs=2, space="PSUM"))
                opsum = ent(tc.tile_pool(name="opsum", bufs=2, space="PSUM"))
                rpsum = ent(tc.tile_pool(name="rpsum", bufs=2, space="PSUM"))
                wpsum = ent(tc.tile_pool(name="wpsum", bufs=2, space="PSUM"))
                gm_pool = (
                    ent(tc.tile_pool(name="gm", bufs=2)) if not causal else None
                )
                wo_sb = wos_pool.tile([128, NDC, D // 2], BF16)
                nc.sync.dma_start(
                    out=wo_sb, in_=wo.rearrange("(o p) f -> p o f", p=128)
                )

                # Big q-chunks first so the final exchange+projection tail is
                # the smallest chunk.
                for qc in (3, 2, 1, 0):
                    nkb = 4 * (qc + 1) if causal else NKB
                    if not causal:
                        gm = gm_pool.tile([128, NKB, QC], BF16, tag="gm")
                        nc.sync.dma_start(
                            out=gm,
                            in_=maskT[:, qc * QC : (qc + 1) * QC].rearrange(
                                "(o p) f -> p o f", p=128
                            ),
                        )
                    for h in range(HG):
                        qh = qh_pool.tile([128, QC], BF16, tag="qh")
                        nc.sync.dma_start(
                            out=qh, in_=q_s[h, :, qc * QC : (qc + 1) * QC]
                        )
                        kh = kh_pool.tile([128, T], BF16, tag="kh")
                        nc.sync.dma_start(
                            out=kh[:, : nkb * 128], in_=k_s[h, :, : nkb * 128]
                        )
                        vh = vh_pool.tile([128, NKB, DV], BF16, tag="vh")
                        nc.sync.dma_start(
                            out=vh[:, :nkb, :],
                            in_=v_s[h, :nkb].rearrange("o p f -> p o f"),
                        )
                        o_ps = opsum.tile([128, QC], F32, tag="opsum")
                        r_ps = rpsum.tile([128, QC], F32, tag="rpsum")
                        for kb in range(nkb):
                            j0 = kb_start(qc, kb)  # first live q col in chunk
                            s_ps = spsum.tile([128, QC], F32, tag="spsum")
                            nc.tensor.matmul(
                                s_ps[:, j0:],
                                lhsT=kh[:, kb * 128 : (kb + 1) * 128],
                                rhs=qh[:, j0:],
                                start=True,
                                stop=True,
                            )
                            pt = pt_pool.tile([128, QC], BF16, tag="pt")
                            nc.scalar.activation(
                                out=pt[:, j0:],
                                in_=s_ps[:, j0:],
                                func=mybir.ActivationFunctionType.Exp,
                                scale=float(SCALE),
                            )
                            if causal:
                                if j0 < QC and kb - 4 * qc >= 0:
                                    # mask the diagonal-crossing 128 cols
                                    nc.vector.tensor_mul(
                                        out=pt[:, j0 : j0 + 128],
                                        in0=pt[:, j0 : j0 + 128],
                                        in1=tri_sb,
                                    )
                            else:
                                nc.vector.tensor_mul(
                                    out=pt, in0=pt, in1=gm[:, kb, :]
                                )
                            nc.tensor.matmul(
                                o_ps[:, j0:],
                                lhsT=vh[:, kb, :],
                                rhs=pt[:, j0:],
                                start=(kb == 0),
                                stop=(kb == nkb - 1),
                            )
                            nc.tensor.matmul(
                                r_ps[:, j0:],
                                lhsT=ones_sb,
                                rhs=pt[:, j0:],
                                start=(kb == 0),
                                stop=(kb == nkb - 1),
                            )
                        rinv = rinv_pool.tile([128, QC], F32, tag="rinv")
                        nc.vector.reciprocal(out=rinv, in_=r_ps)
                        msb = mstage.tile([128, QC], BF16, tag="mstage")
                        nc.vector.tensor_mul(out=msb, in0=o_ps, in1=rinv)
                        nc.sync.dma_start(
                            out=cc_in[qc][h // 2][:, h % 2, :], in_=msb
                        )
                        if h % 2 == 1:
                            nc.gpsimd.collective_compute(
                                "AllGather",
                                mybir.AluOpType.bypass,
                                ins=[cc_in[qc][h // 2][:]],
                                outs=[cc_out[qc][h // 2][:]],
                                replica_groups=pair_groups,
                            )

                    # Output projection for this q-chunk. hv-accumulation is
                    # split in two halves (head-pairs 0,1 then 2,3) so the
                    # first half starts before the last sub-gather lands.
                    mf = []
                    for j in range(4):
                        mfj = mf_pool.tile([128, 4, QC], BF16, tag="mf")
                        nc.sync.dma_start(out=mfj[:, 0:2, :], in_=cc_out[qc][j][0])
                        nc.sync.dma_start(out=mfj[:, 2:4, :], in_=cc_out[qc][j][1])
                        mf.append(mfj)

                    def hv_of(j, t):
                        return 2 * j + t if t < 2 else 8 + 2 * j + (t - 2)

                    parts = []
                    for half in range(2):
                        js = (2 * half, 2 * half + 1)
                        for col in range(D // 2 // 128):
                            w_ps = wpsum.tile([128, QC], F32, tag="wpsum")
                            for j in js:
                                for t in range(4):
                                    nc.tensor.matmul(
                                        w_ps,
                                        lhsT=wo_sb[
                                            :, hv_of(j, t), col * 128 : (col + 1) * 128
                                        ],
                                        rhs=mf[j][:, t, :],
                                        start=(j == js[0] and t == 0),
                                        stop=(j == js[1] and t == 3),
                                    )
                            if half == 0:
                                part = wpart_pool.tile([128, QC], F32, tag="wpart")
                                nc.vector.tensor_copy(out=part, in_=w_ps)
                                parts.append(part)
                            else:
                                ob = ob_pool.tile([128, QC], F32, tag="ob")
                                nc.vector.tensor_add(
                                    out=ob, in0=w_ps, in1=parts[col]
                                )
                                nc.sync.dma_start(
                                    out=outT[
                                        col * 128 : (col + 1) * 128,
                                        qc * QC : (qc + 1) * QC,
                                    ],
                                    in_=ob,
                                )

    nc.compile()
    return nc


def kernel(q, k, v, mask, Wq, Wk, Wv, Wo):
    q = np.asarray(q)
    k = np.asarray(k)
    v = np.asarray(v)
    mask = np.asarray(mask)
    causal = bool(np.array_equal(mask, np.tril(np.ones((T, T), dtype=bool))))

    if causal not in _KERNEL_CACHE:
        _KERNEL_CACHE[causal] = build_kernel(causal)
    nc = _KERNEL_CACHE[causal]

    bf = ml_dtypes.bfloat16
    Wq_b = np.asarray(Wq).astype(bf)
    Wk_b = np.asarray(Wk).astype(bf)
    Wv_b = np.asarray(Wv).astype(bf)
    Wo_b = np.asarray(Wo).astype(bf)
    i = np.arange(128)
    tri_np = (i[None, :] >= i[:, None]).astype(bf)  # tri[k, j] = j >= k
    maskT_np = None if causal else np.ascontiguousarray(mask.T).astype(bf)

    in_maps = []
    for c in range(N_CORES):
        b, g = c // 2, c % 2
        m = {
            "qT": np.ascontiguousarray(q[b].T).astype(bf),
            "kT": np.ascontiguousarray(k[b].T).astype(bf),
            "vT": np.ascontiguousarray(v[b].T).astype(bf),
            "wq": np.ascontiguousarray(Wq_b[:, g * 1024 : (g + 1) * 1024]),
            "wk": np.ascontiguousarray(Wk_b[:, g * 1024 : (g + 1) * 1024]),
            "wv": np.ascontiguousarray(Wv_b[:, g * 1024 : (g + 1) * 1024]),
            "wo": np.ascontiguousarray(Wo_b[:, g * 1024 : (g + 1) * 1024]),
            "tri": tri_np,
        }
        if not causal:
            m["maskT"] = maskT_np
        in_maps.append(m)

    trace = bool(os.environ.get("BASS_KERNEL_TRACE")) and (
        "antenv.axon_hooks" in sys.modules
    )
    res = run_bass_kernel_spmd(nc, in_maps, list(range(N_CORES)), trace=trace)
    if trace and res.exec_time_ns is not None:
        print(f"HW exec time: {res.exec_time_ns} ns")
        kernel.last_exec_time_ns = res.exec_time_ns
        kernel.last_results = res

    out = np.empty((B, T, D), dtype=np.float32)
    for b in range(B):
        top = res.results[2 * b]["outT"]        # cols 0..1023, [1024, 2048]
        bot = res.results[2 * b + 1]["outT"]    # cols 1024..2047
        out[b] = np.concatenate([top, bot], axis=0).T
    return out


# revision 6
# speedup vs baseline: 1.0946x; 1.0045x over previous
"""Distributed multi-head causal attention for 8 TRN2 NeuronCores.

Problem: B=4, T=2048, D=2048, H=16 heads of dk=dv=128.
  out = softmax(mask((q@Wq)(k@Wk)^T / sqrt(dk))) @ (v@Wv) @ Wo

Sharding (2D; all per-core asymmetry lives in host-supplied data so the
SPMD graph is identical on every core):
  core c -> batch b = c//2, head-group g = c%2 (heads 8g..8g+7).
  - QKV projections + attention for (batch b, its 8 heads): fully local.
  - Pair AllGather (replica groups [2b, 2b+1]) exchanges the per-head
    attention outputs (merged^T, bf16), chunked by q-512 AND head-pair
    (16 small collectives) so the exchange pipelines into the output
    projection.
  - Output projection: each core computes out^T for its batch for HALF
    the output columns (even core: cols 0..1023, odd: 1024..2047); the
    hv-accumulation is split in two halves so the first half starts
    before the last sub-AllGather lands.
  Host reassembles: out[b] = concat(outT_2b, outT_2b+1, axis=0).T

Compute is bf16 on TensorE with f32 PSUM accumulation. Softmax skips the
max-subtraction (scores are ~N(0,1); exp is safe in f32) and obtains the
denominators with an extra ones-matmul so everything stays on TensorE;
causal masking multiplies exp(scores) by a 0/1 triangular tile on the
single diagonal-crossing 128x128 sub-block, and the moving free dim of
diagonal-region matmuls is trimmed to the unmasked columns.

Layouts per core (all bf16 unless noted):
  qT/kT/vT [D=2048, T=2048]   x[b].T            (contraction on partitions)
  wq/wk/wv [D=2048, 1024]     W[:, 1024g:1024(g+1)]
  wo       [2048, 1024]       Wo[:, 1024g:1024(g+1)]
  tri      [128, 128]         causal tile, tri[k, j] = j >= k
  maskT    [2048, 2048]       general mode: mask.T (0/1)
  outT     [1024, 2048] f32   out[b][:, cols].T
Internal: q_s/k_s [8, 128, 2048] (Q^T/K^T per head), v_s [8, 16, 128, 128]
  (V natural, per head per k-block), cc_in/cc_out per (q-chunk, head-pair).
"""
import os
import sys
from contextlib import ExitStack

import numpy as np
import ml_dtypes

import concourse.bass as bass
import concourse.mybir as mybir
import concourse.tile as tile
from concourse import bacc
from concourse.bass_utils import run_bass_kernel_spmd

BF16 = mybir.dt.bfloat16
F32 = mybir.dt.float32

B, T, D = 4, 2048, 2048
H, DK, DV = 16, 128, 128
HG = 8                      # heads per core
N_CORES = 8
QC = 512                    # q-chunk (matmul moving free dim)
NQC = T // QC               # 4
NKB = T // 128              # 16 k-blocks
NDC = D // 128              # 16 contraction chunks
SCALE = 1.0 / np.sqrt(DK)
N_WARM = 400                # dummy matmuls to warm the PE clock gate

_KERNEL_CACHE = {}


class _null_ctx:
    def __enter__(self):
        return None

    def __exit__(self, *a):
        return False


def build_kernel(causal: bool):
    nc = bacc.Bacc("TRN2", num_devices=N_CORES)

    qT = nc.declare_dram_parameter("qT", [D, T], BF16, isOutput=False)
    kT = nc.declare_dram_parameter("kT", [D, T], BF16, isOutput=False)
    vT = nc.declare_dram_parameter("vT", [D, T], BF16, isOutput=False)
    wq = nc.declare_dram_parameter("wq", [D, HG * DK], BF16, isOutput=False)
    wk = nc.declare_dram_parameter("wk", [D, HG * DK], BF16, isOutput=False)
    wv = nc.declare_dram_parameter("wv", [D, HG * DV], BF16, isOutput=False)
    wo = nc.declare_dram_parameter("wo", [H * DV, D // 2], BF16, isOutput=False)
    tri = nc.declare_dram_parameter("tri", [128, 128], BF16, isOutput=False)
    if not causal:
        maskT = nc.declare_dram_parameter("maskT", [T, T], BF16, isOutput=False)
    outT = nc.declare_dram_parameter("outT", [D // 2, T], F32, isOutput=True)

    q_s = nc.dram_tensor("q_s", [HG, 128, T], BF16)
    k_s = nc.dram_tensor("k_s", [HG, 128, T], BF16)
    v_s = nc.dram_tensor("v_s", [HG, 128, NKB * DV], BF16)
    # One small collective per (q-chunk, head-pair): cc_in holds the two
    # local heads 2j, 2j+1; the pair gather adds global heads 8+2j, 9+2j.
    cc_in = [
        [nc.dram_tensor(f"cc_in_{qc}_{j}", [128, 2, QC], BF16) for j in range(4)]
        for qc in range(NQC)
    ]
    # 2-core replica groups don't support Shared-output collectives; Local
    # output is the supported path there (see replica_groups.py).
    cc_out = [
        [nc.dram_tensor(f"cc_out_{qc}_{j}", [2, 128, 2, QC], BF16) for j in range(4)]
        for qc in range(NQC)
    ]
    pair_groups = [[0, 1], [2, 3], [4, 5], [6, 7]]

    def kb_start(qc, kb):
        """First unmasked q column (within the chunk) for this k-block."""
        if not causal:
            return 0
        return min(max((kb - 4 * qc) * 128, 0), QC)

    with tile.TileContext(nc) as tc:
        with tc.tile_pool(name="consts", bufs=1) as consts:
            ones_sb = consts.tile([128, 128], BF16)
            nc.vector.memset(ones_sb, 1.0)
            tri_sb = consts.tile([128, 128], BF16)
            nc.sync.dma_start(out=tri_sb, in_=tri[:])

            # Warm the PE HAM clock gate while the first input DMAs land:
            # dependency-free matmuls on the ones tile into a scratch bank.
            with tc.tile_pool(name="warmps", bufs=1, space="PSUM") as warmps:
                wps = warmps.tile([128, 128], F32)
                for i in range(N_WARM):
                    nc.tensor.matmul(
                        wps, lhsT=ones_sb, rhs=ones_sb,
                        start=(i == 0), stop=(i == N_WARM - 1),
                    )

                # ---------------- Phase 1: QKV projections ----------------
                # Q^T/K^T per head ([dk, q], dk on partitions) and V natural
                # ([krows, dv], krows on partitions), staged through DRAM.
                with (
                    tc.tile_pool(name="xt", bufs=4) as xt_pool,
                    tc.tile_pool(name="w", bufs=2) as w_pool,
                    tc.tile_pool(name="pstage", bufs=3) as pstage,
                    tc.tile_pool(name="vstage", bufs=2) as vstage,
                    tc.tile_pool(name="ppsum", bufs=2, space="PSUM") as ppsum,
                    tc.tile_pool(name="vpsum", bufs=2, space="PSUM") as vpsum,
                ):
                    for x_ext, w_ext, kind in (
                        (vT, wv, "v"), (kT, wk, "k"), (qT, wq, "q")
                    ):
                        xh, wh = [], []
                        for half in range(2):
                            xtile = xt_pool.tile([128, NDC // 2, T], BF16, tag="xt")
                            nc.sync.dma_start(
                                out=xtile,
                                in_=x_ext[half * 1024 : (half + 1) * 1024].rearrange(
                                    "(o p) f -> p o f", p=128
                                ),
                            )
                            xh.append(xtile)
                            wtile = w_pool.tile([128, NDC // 2, HG * 128], BF16, tag="w")
                            nc.sync.dma_start(
                                out=wtile,
                                in_=w_ext[half * 1024 : (half + 1) * 1024].rearrange(
                                    "(o p) f -> p o f", p=128
                                ),
                            )
                            wh.append(wtile)

                        if kind in ("q", "k"):
                            dst = q_s if kind == "q" else k_s
                            for h in range(HG):
                                for qc in range(NQC):
                                    ps = ppsum.tile([128, QC], F32, tag="ppsum")
                                    for half in range(2):
                                        for dci in range(NDC // 2):
                                            nc.tensor.matmul(
                                                ps,
                                                lhsT=wh[half][
                                                    :, dci, h * 128 : (h + 1) * 128
                                                ],
                                                rhs=xh[half][
                                                    :, dci, qc * QC : (qc + 1) * QC
                                                ],
                                                start=(half == 0 and dci == 0),
                                                stop=(half == 1 and dci == NDC // 2 - 1),
                                            )
                                    sb = pstage.tile([128, QC], BF16, tag="pstage")
                                    nc.vector.tensor_copy(out=sb, in_=ps)
                                    nc.sync.dma_start(
                                        out=dst[h, :, qc * QC : (qc + 1) * QC], in_=sb
                                    )
                        else:
                            for kb in range(NKB):
                                ps = vpsum.tile([128, HG * DV], F32, tag="vpsum")
                                for half in range(2):
                                    for dci in range(NDC // 2):
                                        lhsT = xh[half][:, dci, kb * 128 : (kb + 1) * 128]
                                        for nn in range(2):
                                            nc.tensor.matmul(
                                                ps[:, nn * 512 : (nn + 1) * 512],
                                                lhsT=lhsT,
                                                rhs=wh[half][
                                                    :, dci, nn * 512 : (nn + 1) * 512
                                                ],
                                                start=(half == 0 and dci == 0),
                                                stop=(half == 1 and dci == NDC // 2 - 1),
                                            )
                                sb = vstage.tile([128, HG * DV], BF16, tag="vstage")
                                nc.vector.tensor_copy(out=sb, in_=ps)
                                for h in range(HG):
                                    nc.sync.dma_start(
                                        out=v_s[h, :, kb * DV : (kb + 1) * DV],
                                        in_=sb[:, h * DV : (h + 1) * DV],
                                    )

            # ---------- Phase 2+3: attention, pair-AG, output proj ----------
            with ExitStack() as phase2:
                ent = phase2.enter_context
                wos_pool = ent(tc.tile_pool(name="wos", bufs=1))
                qh_pool = ent(tc.tile_pool(name="qh", bufs=4))
                kh_pool = ent(tc.tile_pool(name="kh", bufs=4))
                vh_pool = ent(tc.tile_pool(name="vh", bufs=4))
                pt_pool = ent(tc.tile_pool(name="pt", bufs=8))
                mstage = ent(tc.tile_pool(name="mstage", bufs=3))
                rinv_pool = ent(tc.tile_pool(name="rinv", bufs=2))
                mf_pool = ent(tc.tile_pool(name="mf", bufs=8))
                wpart_pool = ent(tc.tile_pool(name="wpart", bufs=8))
                ob_pool = ent(tc.tile_pool(name="ob", bufs=3))
                spsum = ent(tc.tile_pool(name="spsum", bufs=2, space="PSUM"))
                opsum = ent(tc.tile_pool(name="opsum", bufs=2, space="PSUM"))
                rpsum = ent(tc.tile_pool(name="rpsum", bufs=2, space="PSUM"))
                wpsum = ent(tc.tile_pool(name="wpsum", bufs=2, space="PSUM"))
                gm_pool = (
                    ent(tc.tile_pool(name="gm", bufs=2)) if not causal else None
                )
                wo_sb = wos_pool.tile([128, NDC, D // 2], BF16)
                nc.sync.dma_start(
                    out=wo_sb, in_=wo.rearrange("(o p) f -> p o f", p=128)
                )

                # Big q-chunks first so the final exchange+projection tail is
                # the smallest chunk.
                for qc in (3, 2, 1, 0):
                    nkb = 4 * (qc + 1) if causal else NKB
                    if not causal:
                        gm = gm_pool.tile([128, NKB, QC], BF16, tag="gm")
                        nc.sync.dma_start(
                            out=gm,
                            in_=maskT[:, qc * QC : (qc + 1) * QC].rearrange(
                                "(o p) f -> p o f", p=128
                            ),
                        )
                    for h in range(HG):
                        qh = qh_pool.tile([128, QC], BF16, tag="qh")
                        nc.sync.dma_start(
                            out=qh, in_=q_s[h, :, qc * QC : (qc + 1) * QC]
                        )
                        kh = kh_pool.tile([128, T], BF16, tag="kh")
                        nc.sync.dma_start(
                            out=kh[:, : nkb * 128], in_=k_s[h, :, : nkb * 128]
                        )
                        vh = vh_pool.tile([128, NKB * DV], BF16, tag="vh")
                        nc.sync.dma_start(
                            out=vh[:, : nkb * DV], in_=v_s[h, :, : nkb * DV]
                        )
                        o_ps = opsum.tile([128, QC], F32, tag="opsum")
                        r_ps = rpsum.tile([128, QC], F32, tag="rpsum")
                        for kb in range(nkb):
                            j0 = kb_start(qc, kb)  # first live q col in chunk
                            s_ps = spsum.tile([128, QC], F32, tag="spsum")
                            nc.tensor.matmul(
                                s_ps[:, j0:],
                                lhsT=kh[:, kb * 128 : (kb + 1) * 128],
                                rhs=qh[:, j0:],
                                start=True,
                                stop=True,
                            )
                            pt = pt_pool.tile([128, QC], BF16, tag="pt")
                            nc.scalar.activation(
                                out=pt[:, j0:],
                                in_=s_ps[:, j0:],
                                func=mybir.ActivationFunctionType.Exp,
                                scale=float(SCALE),
                            )
                            if causal:
                                if j0 < QC and kb - 4 * qc >= 0:
                                    # mask the diagonal-crossing 128 cols
                                    nc.vector.tensor_mul(
                                        out=pt[:, j0 : j0 + 128],
                                        in0=pt[:, j0 : j0 + 128],
                                        in1=tri_sb,
                                    )
                            else:
                                nc.vector.tensor_mul(
                                    out=pt, in0=pt, in1=gm[:, kb, :]
                                )
                            nc.tensor.matmul(
                                o_ps[:, j0:],
                                lhsT=vh[:, kb * DV : (kb + 1) * DV],
                                rhs=pt[:, j0:],
                                start=(kb == 0),
                                stop=(kb == nkb - 1),
                            )
                            nc.tensor.matmul(
                                r_ps[:, j0:],
                                lhsT=ones_sb,
                                rhs=pt[:, j0:],
                                start=(kb == 0),
                                stop=(kb == nkb - 1),
                            )
                        rinv = rinv_pool.tile([128, QC], F32, tag="rinv")
                        nc.vector.reciprocal(out=rinv, in_=r_ps)
                        msb = mstage.tile([128, QC], BF16, tag="mstage")
                        nc.vector.tensor_mul(out=msb, in0=o_ps, in1=rinv)
                        nc.sync.dma_start(
                            out=cc_in[qc][h // 2][:, h % 2, :], in_=msb
                        )
                        if h % 2 == 1:
                            nc.gpsimd.collective_compute(
                                "AllGather",
                                mybir.AluOpType.bypass,
                                ins=[cc_in[qc][h // 2][:]],
                                outs=[cc_out[qc][h // 2][:]],
                                replica_groups=pair_groups,
                            )

                    # Output projection for this q-chunk. hv-accumulation is
                    # split in two halves (head-pairs 0,1 then 2,3) so the
                    # first half starts before the last sub-gather lands.
                    mf = []
                    for j in range(4):
                        mfj = mf_pool.tile([128, 4, QC], BF16, tag="mf")
                        nc.sync.dma_start(out=mfj[:, 0:2, :], in_=cc_out[qc][j][0])
                        nc.sync.dma_start(out=mfj[:, 2:4, :], in_=cc_out[qc][j][1])
                        mf.append(mfj)

                    def hv_of(j, t):
                        return 2 * j + t if t < 2 else 8 + 2 * j + (t - 2)

                    parts = []
                    for half in range(2):
                        js = (2 * half, 2 * half + 1)
                        for col in range(D // 2 // 128):
                            w_ps = wpsum.tile([128, QC], F32, tag="wpsum")
                            for j in js:
                                for t in range(4):
                                    nc.tensor.matmul(
                                        w_ps,
                                        lhsT=wo_sb[
                                            :, hv_of(j, t), col * 128 : (col + 1) * 128
                                        ],
                                        rhs=mf[j][:, t, :],
                                        start=(j == js[0] and t == 0),
                                        stop=(j == js[1] and t == 3),
                                    )
                            if half == 0:
                                part = wpart_pool.tile([128, QC], F32, tag="wpart")
                                nc.vector.tensor_copy(out=part, in_=w_ps)
                                parts.append(part)
                            else:
                                ob = ob_pool.tile([128, QC], F32, tag="ob")
                                nc.vector.tensor_add(
                                    out=ob, in0=w_ps, in1=parts[col]
                                )
                                nc.sync.dma_start(
                                    out=outT[
                                        col * 128 : (col + 1) * 128,
                                        qc * QC : (qc + 1) * QC,
                                    ],
                                    in_=ob,
                                )

    nc.compile()
    return nc


def kernel(q, k, v, mask, Wq, Wk, Wv, Wo):
    q = np.asarray(q)
    k = np.asarray(k)
    v = np.asarray(v)
    mask = np.asarray(mask)
    causal = bool(np.array_equal(mask, np.tril(np.ones((T, T), dtype=bool))))

    if causal not in _KERNEL_CACHE:
        _KERNEL_CACHE[causal] = build_kernel(causal)
    nc = _KERNEL_CACHE[causal]

    bf = ml_dtypes.bfloat16
    Wq_b = np.asarray(Wq).astype(bf)
    Wk_b = np.asarray(Wk).astype(bf)
    Wv_b = np.asarray(Wv).astype(bf)
    Wo_b = np.asarray(Wo).astype(bf)
    i = np.arange(128)
    tri_np = (i[None, :] >= i[:, None]).astype(bf)  # tri[k, j] = j >= k
    maskT_np = None if causal else np.ascontiguousarray(mask.T).astype(bf)

    in_maps = []
    for c in range(N_CORES):
        b, g = c // 2, c % 2
        m = {
            "qT": np.ascontiguousarray(q[b].T).astype(bf),
            "kT": np.ascontiguousarray(k[b].T).astype(bf),
            "vT": np.ascontiguousarray(v[b].T).astype(bf),
            "wq": np.ascontiguousarray(Wq_b[:, g * 1024 : (g + 1) * 1024]),
            "wk": np.ascontiguousarray(Wk_b[:, g * 1024 : (g + 1) * 1024]),
            "wv": np.ascontiguousarray(Wv_b[:, g * 1024 : (g + 1) * 1024]),
            "wo": np.ascontiguousarray(Wo_b[:, g * 1024 : (g + 1) * 1024]),
            "tri": tri_np,
        }
        if not causal:
            m["maskT"] = maskT_np
        in_maps.append(m)

    trace = bool(os.environ.get("BASS_KERNEL_TRACE")) and (
        "antenv.axon_hooks" in sys.modules
    )
    res = run_bass_kernel_spmd(nc, in_maps, list(range(N_CORES)), trace=trace)
    if trace and res.exec_time_ns is not None:
        print(f"HW exec time: {res.exec_time_ns} ns")
        kernel.last_exec_time_ns = res.exec_time_ns
        kernel.last_results = res

    out = np.empty((B, T, D), dtype=np.float32)
    for b in range(B):
        top = res.results[2 * b]["outT"]        # cols 0..1023, [1024, 2048]
        bot = res.results[2 * b + 1]["outT"]    # cols 1024..2047
        out[b] = np.concatenate([top, bot], axis=0).T
    return out
